# revision 16
# baseline (speedup 1.0000x reference)
"""Trainium2 Bass kernel for nn_ContinuousPool.

Computes, for x:(32,96,128,128) f32 and pool_strength:(1,96,1,1) f32:
    cur = x
    repeat 10: cur = cur + s * (maxpool3x3_same(cur) - cur)
    out = avgpool2x2(cur)            -> (32,96,64,64)

Strategy (v2, fp16):
  - Pure data parallel over 8 cores: 384 images/core, processed as 3
    chunks of 128 images (one image per SBUF partition).
  - The blend is rescaled to eliminate one multiply:
        u_t = cur_t / (1-s)^t  evolves as  u' = u + (s/(1-s)) * maxpool(u)
    and the final avgpool folds the (1-s)^10 / 4 factor into one scale.
  - All evolution tensors are fp16: tensor_tensor max/add hit the DVE
    2x_1p perf mode (2 elem/cycle, ~9us per 16K-elem op vs ~17.5us fp32).
  - Per step: 2 TT row-max + 2 TT col-max + TT add on DVE; the c*w scale
    runs on the Scalar engine (hidden) for the two interleaved chunks.
    Chunks A,B are step-interleaved so ScalarE latency is off the DVE
    critical path; the last chunk C runs solo with a DVE tensor_scalar
    scale (4x mode) instead.
  - Frames are padded to 130x130 with -60000 on the host (pads DMA in
    with the data); r keeps static -60000 pad rows so no per-step pad
    maintenance is needed. Interior ops never touch pads.
"""

import os
import sys

import numpy as np

if "/opt/trn_rl_repo" not in sys.path:
    sys.path.insert(0, "/opt/trn_rl_repo")

B, C, H, W = 32, 96, 128, 128
T = 10
N_CORES = 8
B_PER_CORE = B // N_CORES          # 4
IMGS = B_PER_CORE * C              # 384 images per core
CHUNK = 128                        # images (partitions) per chunk
NCHUNK = IMGS // CHUNK             # 3
HP = WP = 130                      # padded frame
FRAME = HP * WP                    # 16900
FRAME_A = FRAME + 4                # tile alloc, room for shifted views
NEG = -60000.0                     # pad value (fp16-representable)

USE_GP_ADD = os.environ.get("USE_GP_ADD", "1") == "1"
SCALE_MODE = os.environ.get("SCALE_MODE", "scalar")  # scalar | dve
C_SPLIT = int(os.environ.get("C_SPLIT", "3"))  # row-splits for the solo chunk
FINISH_GP = os.environ.get("FINISH_GP", "0") == "1"

_CACHE = {}


def _build_program(loop_reps=1):
    import contextlib
    import concourse.bacc as bacc
    import concourse.mybir as mybir
    from concourse import tile

    f32 = mybir.dt.float32
    f16 = mybir.dt.float16
    alu = mybir.AluOpType
    act = mybir.ActivationFunctionType

    nc = bacc.Bacc("TRN2", target_bir_lowering=False, debug=False,
                   num_devices=N_CORES)

    x_d = nc.dram_tensor("x", [IMGS, FRAME], f16, kind="ExternalInput")
    c_d = nc.dram_tensor("cvec", [IMGS, 1], f32, kind="ExternalInput")
    f_d = nc.dram_tensor("fvec", [IMGS, 1], f32, kind="ExternalInput")
    y_d = nc.dram_tensor("y", [IMGS, (H // 2) * (W // 2)], f16,
                         kind="ExternalOutput")

    with tile.TileContext(nc, num_cores=N_CORES) as tc:
        with tc.tile_pool(name="main", bufs=1) as pool:
            # two pipeline slots (A/B); chunk 2 reuses slot 0
            u_ts = [pool.tile([128, FRAME_A], f16, tag=f"u{i}", name=f"u{i}")
                    for i in (0, 1)]
            r_ts = [pool.tile([128, HP * W + 4], f16, tag=f"r{i}", name=f"r{i}")
                    for i in (0, 1)]
            w_ts = [pool.tile([128, H * W], f16, tag=f"w{i}", name=f"w{i}")
                    for i in (0, 1)]
            cs_ts = [pool.tile([128, 2], f32, tag=f"cs{i}", name=f"cs{i}")
                     for i in (0, 1)]

            # r pad rows (row 0 and row 129) are static -60000; interior
            # rows 1..128 are fully rewritten each step.
            for r_t in r_ts:
                nc.vector.memset(r_t[:, 0:W], NEG)
                nc.vector.memset(r_t[:, (HP - 1) * W:HP * W], NEG)

            def u3(i):
                return u_ts[i][:, 0:FRAME].rearrange(
                    "p (h w) -> p h w", h=HP, w=WP)

            def r3(i):
                return r_ts[i][:, 0:HP * W].rearrange(
                    "p (h w) -> p h w", h=HP, w=W)

            def w3(i):
                return w_ts[i][:, :].rearrange("p (h w) -> p h w", h=H, w=W)

            def load(k, slot):
                rows = slice(k * CHUNK, (k + 1) * CHUNK)
                x_v = x_d[rows, :].rearrange("p (h w) -> p h w", h=HP, w=WP)
                uv = u3(slot)
                nc.sync.dma_start(uv[:, 0:65, :], x_v[:, 0:65, :])
                nc.sync.dma_start(uv[:, 65:130, :], x_v[:, 65:130, :])
                nc.sync.dma_start(cs_ts[slot][:, 0:1], c_d[rows, :])
                nc.sync.dma_start(cs_ts[slot][:, 1:2], f_d[rows, :])

            def step(slot, scale_on_scalar):
                uv, rv, wv = u3(slot), r3(slot), w3(slot)
                u_int = uv[:, 1:129, 1:129]
                r_int = rv[:, 1:129, :]
                # row max3 (reads u col pads; writes r interior rows)
                nc.vector.tensor_max(r_int, uv[:, 1:129, 0:128],
                                     uv[:, 1:129, 2:130])
                nc.vector.tensor_max(r_int, r_int, u_int)
                # col max3 (reads r pad rows; writes w)
                nc.vector.tensor_max(wv, rv[:, 0:128, :], rv[:, 2:130, :])
                nc.vector.tensor_max(wv, wv, r_int)
                # blend: u += c * w  (scale off the DVE critical path when
                # a partner chunk's maxes can run meanwhile)
                if scale_on_scalar and SCALE_MODE == "scalar":
                    nc.scalar.activation(w_ts[slot][:, :], w_ts[slot][:, :],
                                         act.Copy, scale=cs_ts[slot][:, 0:1])
                else:
                    nc.vector.tensor_scalar_mul(w_ts[slot][:, :],
                                                w_ts[slot][:, :],
                                                cs_ts[slot][:, 0:1])
                if USE_GP_ADD and scale_on_scalar:
                    nc.gpsimd.tensor_add(u_int, u_int, wv)
                else:
                    nc.vector.tensor_add(u_int, u_int, wv)

            def step_solo(slot):
                # Solo chunk: no partner work to hide ScalarE/gpsimd latency
                # behind, so split rows 3-ways and pipeline the thirds:
                # DVE runs max ops back-to-back while each third's scale
                # (ScalarE) and blend add (gpsimd) trail behind.
                uv, rv, wv = u3(slot), r3(slot), w3(slot)
                bounds = [1 + (128 * i) // C_SPLIT for i in range(C_SPLIT + 1)]
                parts = list(zip(bounds[:-1], bounds[1:]))

                def rowmax(lo, hi):
                    r_p = rv[:, lo:hi, :]
                    nc.vector.tensor_max(r_p, uv[:, lo:hi, 0:128],
                                         uv[:, lo:hi, 2:130])
                    nc.vector.tensor_max(r_p, r_p, uv[:, lo:hi, 1:129])

                def colmax_blend(lo, hi):
                    w_p = wv[:, lo - 1:hi - 1, :]
                    nc.vector.tensor_max(w_p, rv[:, lo - 1:hi - 1, :],
                                         rv[:, lo + 1:hi + 1, :])
                    nc.vector.tensor_max(w_p, w_p, rv[:, lo:hi, :])
                    if USE_GP_ADD and SCALE_MODE == "scalar":
                        nc.scalar.activation(w_p, w_p, act.Copy,
                                             scale=cs_ts[slot][:, 0:1])
                        nc.gpsimd.tensor_add(uv[:, lo:hi, 1:129],
                                             uv[:, lo:hi, 1:129], w_p)
                    else:
                        nc.vector.tensor_scalar_mul(w_p, w_p,
                                                    cs_ts[slot][:, 0:1])
                        nc.vector.tensor_add(uv[:, lo:hi, 1:129],
                                             uv[:, lo:hi, 1:129], w_p)

                # interleave: col max of part i needs row max of part i+1
                rowmax(*parts[0])
                for i in range(1, C_SPLIT):
                    rowmax(*parts[i])
                    colmax_blend(*parts[i - 1])
                colmax_blend(*parts[-1])

            def finish(k, slot, on_gp=False):
                # avgpool 2x2 + per-channel scale, result into w slot
                eng = nc.gpsimd if on_gp else nc.vector
                rows = slice(k * CHUNK, (k + 1) * CHUNK)
                uv, rv, wv = u3(slot), r3(slot), w3(slot)
                # horizontal pairs: q[h, j] = u[h, 2j+1] + u[h, 2j+2]
                u4 = u_ts[slot][:, WP:WP + H * WP].rearrange(
                    "p (h w2 two) -> p h w2 two", h=H, w2=WP // 2, two=2)
                q = rv[:, 1:129, 0:64]
                eng.tensor_add(q, u4[:, :, 0:64, 1:2], u4[:, :, 1:65, 0:1])
                # vertical pairs
                q5 = r_ts[slot][:, W:W + H * W].rearrange(
                    "p (h2 two w) -> p h2 two w", h2=H // 2, two=2, w=W)
                z = wv[:, 0:64, 0:64]
                eng.tensor_add(z, q5[:, :, 0:1, 0:64], q5[:, :, 1:2, 0:64])
                nc.vector.tensor_scalar_mul(z, z, cs_ts[slot][:, 1:2])
                nc.sync.dma_start(
                    y_d[rows, :].rearrange("p (h w) -> p h w", h=64, w=64), z)

            loop_cm = (tc.For_i(0, loop_reps) if loop_reps > 1
                       else contextlib.nullcontext())
            with loop_cm:
                load(0, 0)
                load(1, 1)
                for _ in range(T):
                    step(0, True)
                    step(1, True)
                finish(0, 0, on_gp=FINISH_GP)
                load(2, 0)
                if C_SPLIT > 1:
                    for _ in range(T):
                        step_solo(0)
                else:
                    for _ in range(T):
                        step(0, False)
                finish(1, 1, on_gp=FINISH_GP)
                finish(2, 0)

    nc.compile()
    return nc


def _get_program():
    if "nc" not in _CACHE:
        _CACHE["nc"] = _build_program()
    return _CACHE["nc"]


def pack_inputs(x, pool_strength):
    """Host-side prep: per-core fp16 padded frames + per-channel scales."""
    x = np.asarray(x, dtype=np.float32)
    s = np.asarray(pool_strength, dtype=np.float64).reshape(C)
    c_ch = (s / (1.0 - s)).astype(np.float32)                  # [C]
    f_ch = (((1.0 - s) ** T) * 0.25).astype(np.float32)        # [C]
    cvec = np.ascontiguousarray(np.tile(c_ch, B_PER_CORE)[:, None])  # [384,1]
    fvec = np.ascontiguousarray(np.tile(f_ch, B_PER_CORE)[:, None])

    in_maps = []
    for j in range(N_CORES):
        xj = x[j * B_PER_CORE:(j + 1) * B_PER_CORE].reshape(IMGS, H, W)
        pad = np.full((IMGS, HP, WP), NEG, dtype=np.float16)
        pad[:, 1:129, 1:129] = xj.astype(np.float16)
        in_maps.append({"x": np.ascontiguousarray(pad.reshape(IMGS, FRAME)),
                        "cvec": cvec, "fvec": fvec})
    return in_maps


def kernel(x: np.ndarray, pool_strength: np.ndarray) -> np.ndarray:
    from concourse.bass_utils import run_bass_kernel_spmd

    nc = _get_program()
    in_maps = pack_inputs(x, pool_strength)
    res = run_bass_kernel_spmd(nc, in_maps, list(range(N_CORES)))

    out = np.empty((B, C, H // 2, W // 2), dtype=np.float32)
    for j in range(N_CORES):
        yj = res.results[j]["y"].astype(np.float32).reshape(
            B_PER_CORE, C, H // 2, W // 2)
        out[j * B_PER_CORE:(j + 1) * B_PER_CORE] = yj
    return out


# revision 21
# speedup vs baseline: 1.0937x; 1.0937x over previous
"""Trainium2 Bass kernel for nn_ContinuousPool.

Computes, for x:(32,96,128,128) f32 and pool_strength:(1,96,1,1) f32:
    cur = x
    repeat 10: cur = cur + s * (maxpool3x3_same(cur) - cur)
    out = avgpool2x2(cur)            -> (32,96,64,64)

Strategy (v2, fp16):
  - Pure data parallel over 8 cores: 384 images/core, processed as 3
    chunks of 128 images (one image per SBUF partition).
  - The blend is rescaled to eliminate one multiply:
        u_t = cur_t / (1-s)^t  evolves as  u' = u + (s/(1-s)) * maxpool(u)
    and the final avgpool folds the (1-s)^10 / 4 factor into one scale.
  - All evolution tensors are fp16: tensor_tensor max/add hit the DVE
    2x_1p perf mode (2 elem/cycle, ~9us per 16K-elem op vs ~17.5us fp32).
  - Per step: 2 TT row-max + 2 TT col-max + TT add on DVE; the c*w scale
    runs on the Scalar engine (hidden) for the two interleaved chunks.
    Chunks A,B are step-interleaved so ScalarE latency is off the DVE
    critical path; the last chunk C runs solo with a DVE tensor_scalar
    scale (4x mode) instead.
  - Frames are padded to 130x130 with -60000 on the host (pads DMA in
    with the data); r keeps static -60000 pad rows so no per-step pad
    maintenance is needed. Interior ops never touch pads.
"""

import os
import sys

import numpy as np

if "/opt/trn_rl_repo" not in sys.path:
    sys.path.insert(0, "/opt/trn_rl_repo")

B, C, H, W = 32, 96, 128, 128
T = 10
N_CORES = 8
B_PER_CORE = B // N_CORES          # 4
IMGS = B_PER_CORE * C              # 384 images per core
CHUNK = 128                        # images (partitions) per chunk
NCHUNK = IMGS // CHUNK             # 3
HP = WP = 130                      # padded frame
FRAME = HP * WP                    # 16900
FRAME_A = FRAME + 4                # tile alloc, room for shifted views
NEG = -60000.0                     # pad value (fp16-representable)

USE_GP_ADD = os.environ.get("USE_GP_ADD", "1") == "1"
SCALE_MODE = os.environ.get("SCALE_MODE", "scalar")  # scalar | dve
C_SPLIT = int(os.environ.get("C_SPLIT", "3"))  # row-splits for the solo chunk
FINISH_GP = os.environ.get("FINISH_GP", "0") == "1"

_CACHE = {}


def _build_program(loop_reps=1, timing_small=False):
    import contextlib
    import concourse.bacc as bacc
    import concourse.mybir as mybir
    from concourse import tile

    f32 = mybir.dt.float32
    f16 = mybir.dt.float16
    alu = mybir.AluOpType
    act = mybir.ActivationFunctionType

    nc = bacc.Bacc("TRN2", target_bir_lowering=False, debug=False,
                   num_devices=N_CORES)

    # timing_small: 1-chunk DRAM footprint (all chunks read/write the same
    # rows) so host<->device transfer noise doesn't pollute slope timing;
    # on-device work is identical.
    n_io = CHUNK if timing_small else IMGS
    x_d = nc.dram_tensor("x", [n_io, FRAME], f16, kind="ExternalInput")
    c_d = nc.dram_tensor("cvec", [IMGS, 1], f32, kind="ExternalInput")
    f_d = nc.dram_tensor("fvec", [IMGS, 1], f32, kind="ExternalInput")
    y_d = nc.dram_tensor("y", [n_io, (H // 2) * (W // 2)], f16,
                         kind="ExternalOutput")

    with tile.TileContext(nc, num_cores=N_CORES) as tc:
        with tc.tile_pool(name="main", bufs=1) as pool:
            # two pipeline slots (A/B); chunk 2 reuses slot 0
            u_ts = [pool.tile([128, FRAME_A], f16, tag=f"u{i}", name=f"u{i}")
                    for i in (0, 1)]
            r_ts = [pool.tile([128, HP * W + 4], f16, tag=f"r{i}", name=f"r{i}")
                    for i in (0, 1)]
            w_ts = [pool.tile([128, H * W], f16, tag=f"w{i}", name=f"w{i}")
                    for i in (0, 1)]
            cs_ts = [pool.tile([128, 2], f32, tag=f"cs{i}", name=f"cs{i}")
                     for i in (0, 1)]

            # r pad rows (row 0 and row 129) are static -60000; interior
            # rows 1..128 are fully rewritten each step.
            for r_t in r_ts:
                nc.vector.memset(r_t[:, 0:W], NEG)
                nc.vector.memset(r_t[:, (HP - 1) * W:HP * W], NEG)

            def u3(i):
                return u_ts[i][:, 0:FRAME].rearrange(
                    "p (h w) -> p h w", h=HP, w=WP)

            def r3(i):
                return r_ts[i][:, 0:HP * W].rearrange(
                    "p (h w) -> p h w", h=HP, w=W)

            def w3(i):
                return w_ts[i][:, :].rearrange("p (h w) -> p h w", h=H, w=W)

            def load(k, slot):
                if timing_small:
                    k = 0
                rows = slice(k * CHUNK, (k + 1) * CHUNK)
                x_v = x_d[rows, :].rearrange("p (h w) -> p h w", h=HP, w=WP)
                uv = u3(slot)
                nc.sync.dma_start(uv[:, 0:65, :], x_v[:, 0:65, :])
                nc.sync.dma_start(uv[:, 65:130, :], x_v[:, 65:130, :])
                nc.sync.dma_start(cs_ts[slot][:, 0:1], c_d[rows, :])
                nc.sync.dma_start(cs_ts[slot][:, 1:2], f_d[rows, :])

            def step(slot, scale_on_scalar):
                uv, rv, wv = u3(slot), r3(slot), w3(slot)
                u_int = uv[:, 1:129, 1:129]
                r_int = rv[:, 1:129, :]
                # row max3 (reads u col pads; writes r interior rows)
                nc.vector.tensor_max(r_int, uv[:, 1:129, 0:128],
                                     uv[:, 1:129, 2:130])
                nc.vector.tensor_max(r_int, r_int, u_int)
                # col max3 (reads r pad rows; writes w)
                nc.vector.tensor_max(wv, rv[:, 0:128, :], rv[:, 2:130, :])
                nc.vector.tensor_max(wv, wv, r_int)
                # blend: u += c * w  (scale off the DVE critical path when
                # a partner chunk's maxes can run meanwhile)
                if scale_on_scalar and SCALE_MODE == "scalar":
                    nc.scalar.activation(w_ts[slot][:, :], w_ts[slot][:, :],
                                         act.Copy, scale=cs_ts[slot][:, 0:1])
                else:
                    nc.vector.tensor_scalar_mul(w_ts[slot][:, :],
                                                w_ts[slot][:, :],
                                                cs_ts[slot][:, 0:1])
                if USE_GP_ADD and scale_on_scalar:
                    nc.gpsimd.tensor_add(u_int, u_int, wv)
                else:
                    nc.vector.tensor_add(u_int, u_int, wv)

            def step_solo(slot):
                # Solo chunk: no partner work to hide ScalarE/gpsimd latency
                # behind, so split rows 3-ways and pipeline the thirds:
                # DVE runs max ops back-to-back while each third's scale
                # (ScalarE) and blend add (gpsimd) trail behind.
                uv, rv, wv = u3(slot), r3(slot), w3(slot)
                bounds = [1 + (128 * i) // C_SPLIT for i in range(C_SPLIT + 1)]
                parts = list(zip(bounds[:-1], bounds[1:]))

                def rowmax(lo, hi):
                    r_p = rv[:, lo:hi, :]
                    nc.vector.tensor_max(r_p, uv[:, lo:hi, 0:128],
                                         uv[:, lo:hi, 2:130])
                    nc.vector.tensor_max(r_p, r_p, uv[:, lo:hi, 1:129])

                def colmax_blend(lo, hi):
                    w_p = wv[:, lo - 1:hi - 1, :]
                    nc.vector.tensor_max(w_p, rv[:, lo - 1:hi - 1, :],
                                         rv[:, lo + 1:hi + 1, :])
                    nc.vector.tensor_max(w_p, w_p, rv[:, lo:hi, :])
                    if USE_GP_ADD and SCALE_MODE == "scalar":
                        nc.scalar.activation(w_p, w_p, act.Copy,
                                             scale=cs_ts[slot][:, 0:1])
                        nc.gpsimd.tensor_add(uv[:, lo:hi, 1:129],
                                             uv[:, lo:hi, 1:129], w_p)
                    else:
                        nc.vector.tensor_scalar_mul(w_p, w_p,
                                                    cs_ts[slot][:, 0:1])
                        nc.vector.tensor_add(uv[:, lo:hi, 1:129],
                                             uv[:, lo:hi, 1:129], w_p)

                # interleave: col max of part i needs row max of part i+1
                rowmax(*parts[0])
                for i in range(1, C_SPLIT):
                    rowmax(*parts[i])
                    colmax_blend(*parts[i - 1])
                colmax_blend(*parts[-1])

            def finish(k, slot, on_gp=False):
                # avgpool 2x2 + per-channel scale, result into w slot
                eng = nc.gpsimd if on_gp else nc.vector
                if timing_small:
                    k = 0
                rows = slice(k * CHUNK, (k + 1) * CHUNK)
                uv, rv, wv = u3(slot), r3(slot), w3(slot)
                # horizontal pairs: q[h, j] = u[h, 2j+1] + u[h, 2j+2]
                u4 = u_ts[slot][:, WP:WP + H * WP].rearrange(
                    "p (h w2 two) -> p h w2 two", h=H, w2=WP // 2, two=2)
                q = rv[:, 1:129, 0:64]
                eng.tensor_add(q, u4[:, :, 0:64, 1:2], u4[:, :, 1:65, 0:1])
                # vertical pairs
                q5 = r_ts[slot][:, W:W + H * W].rearrange(
                    "p (h2 two w) -> p h2 two w", h2=H // 2, two=2, w=W)
                z = wv[:, 0:64, 0:64]
                eng.tensor_add(z, q5[:, :, 0:1, 0:64], q5[:, :, 1:2, 0:64])
                nc.vector.tensor_scalar_mul(z, z, cs_ts[slot][:, 1:2])
                nc.sync.dma_start(
                    y_d[rows, :].rearrange("p (h w) -> p h w", h=64, w=64), z)

            loop_cm = (tc.For_i(0, loop_reps) if loop_reps > 1
                       else contextlib.nullcontext())
            with loop_cm:
                load(0, 0)
                load(1, 1)
                for _ in range(T):
                    step(0, True)
                    step(1, True)
                finish(0, 0, on_gp=FINISH_GP)
                load(2, 0)
                if C_SPLIT > 1:
                    for _ in range(T):
                        step_solo(0)
                else:
                    for _ in range(T):
                        step(0, False)
                finish(1, 1, on_gp=FINISH_GP)
                finish(2, 0)

    nc.compile()
    return nc


def _get_program():
    if "nc" not in _CACHE:
        _CACHE["nc"] = _build_program()
    return _CACHE["nc"]


def pack_inputs(x, pool_strength):
    """Host-side prep: per-core fp16 padded frames + per-channel scales."""
    x = np.asarray(x, dtype=np.float32)
    s = np.asarray(pool_strength, dtype=np.float64).reshape(C)
    c_ch = (s / (1.0 - s)).astype(np.float32)                  # [C]
    f_ch = (((1.0 - s) ** T) * 0.25).astype(np.float32)        # [C]
    cvec = np.ascontiguousarray(np.tile(c_ch, B_PER_CORE)[:, None])  # [384,1]
    fvec = np.ascontiguousarray(np.tile(f_ch, B_PER_CORE)[:, None])

    in_maps = []
    for j in range(N_CORES):
        xj = x[j * B_PER_CORE:(j + 1) * B_PER_CORE].reshape(IMGS, H, W)
        pad = np.full((IMGS, HP, WP), NEG, dtype=np.float16)
        pad[:, 1:129, 1:129] = xj.astype(np.float16)
        in_maps.append({"x": np.ascontiguousarray(pad.reshape(IMGS, FRAME)),
                        "cvec": cvec, "fvec": fvec})
    return in_maps


def kernel(x: np.ndarray, pool_strength: np.ndarray) -> np.ndarray:
    from concourse.bass_utils import run_bass_kernel_spmd

    nc = _get_program()
    in_maps = pack_inputs(x, pool_strength)
    res = run_bass_kernel_spmd(nc, in_maps, list(range(N_CORES)))

    out = np.empty((B, C, H // 2, W // 2), dtype=np.float32)
    for j in range(N_CORES):
        yj = res.results[j]["y"].astype(np.float32).reshape(
            B_PER_CORE, C, H // 2, W // 2)
        out[j * B_PER_CORE:(j + 1) * B_PER_CORE] = yj
    return out


# revision 22
# speedup vs baseline: 1.5617x; 1.4279x over previous
"""Trainium2 Bass kernel for nn_ContinuousPool.

Computes, for x:(32,96,128,128) f32 and pool_strength:(1,96,1,1) f32:
    cur = x
    repeat 10: cur = cur + s * (maxpool3x3_same(cur) - cur)
    out = avgpool2x2(cur)            -> (32,96,64,64)

Strategy (v2, fp16):
  - Pure data parallel over 8 cores: 384 images/core, processed as 3
    chunks of 128 images (one image per SBUF partition).
  - The blend is rescaled to eliminate one multiply:
        u_t = cur_t / (1-s)^t  evolves as  u' = u + (s/(1-s)) * maxpool(u)
    and the final avgpool folds the (1-s)^10 / 4 factor into one scale.
  - All evolution tensors are fp16: tensor_tensor max/add hit the DVE
    2x_1p perf mode (2 elem/cycle, ~9us per 16K-elem op vs ~17.5us fp32).
  - Per step: 2 TT row-max + 2 TT col-max + TT add on DVE; the c*w scale
    runs on the Scalar engine (hidden) for the two interleaved chunks.
    Chunks A,B are step-interleaved so ScalarE latency is off the DVE
    critical path; the last chunk C runs solo with a DVE tensor_scalar
    scale (4x mode) instead.
  - Frames are padded to 130x130 with -60000 on the host (pads DMA in
    with the data); r keeps static -60000 pad rows so no per-step pad
    maintenance is needed. Interior ops never touch pads.
"""

import os
import sys

import numpy as np

if "/opt/trn_rl_repo" not in sys.path:
    sys.path.insert(0, "/opt/trn_rl_repo")

B, C, H, W = 32, 96, 128, 128
T = 10
N_CORES = 8
B_PER_CORE = B // N_CORES          # 4
IMGS = B_PER_CORE * C              # 384 images per core
CHUNK = 128                        # images (partitions) per chunk
NCHUNK = IMGS // CHUNK             # 3
HP = WP = 130                      # padded frame
FRAME = HP * WP                    # 16900
FRAME_A = FRAME + 4                # tile alloc, room for shifted views
NEG = -60000.0                     # pad value (fp16-representable)

USE_GP_ADD = os.environ.get("USE_GP_ADD", "0") == "1"
SCALE_MODE = os.environ.get("SCALE_MODE", "scalar")  # scalar | dve
C_SPLIT = int(os.environ.get("C_SPLIT", "1"))  # row-splits for the solo chunk
FINISH_GP = os.environ.get("FINISH_GP", "0") == "1"

_CACHE = {}


def _build_program(loop_reps=1, timing_small=False):
    import contextlib
    import concourse.bacc as bacc
    import concourse.mybir as mybir
    from concourse import tile

    f32 = mybir.dt.float32
    f16 = mybir.dt.float16
    alu = mybir.AluOpType
    act = mybir.ActivationFunctionType

    nc = bacc.Bacc("TRN2", target_bir_lowering=False, debug=False,
                   num_devices=N_CORES)

    # timing_small: 1-chunk DRAM footprint (all chunks read/write the same
    # rows) so host<->device transfer noise doesn't pollute slope timing;
    # on-device work is identical.
    n_io = CHUNK if timing_small else IMGS
    x_d = nc.dram_tensor("x", [n_io, FRAME], f16, kind="ExternalInput")
    c_d = nc.dram_tensor("cvec", [IMGS, 1], f32, kind="ExternalInput")
    f_d = nc.dram_tensor("fvec", [IMGS, 1], f32, kind="ExternalInput")
    y_d = nc.dram_tensor("y", [n_io, (H // 2) * (W // 2)], f16,
                         kind="ExternalOutput")

    with tile.TileContext(nc, num_cores=N_CORES) as tc:
        with tc.tile_pool(name="main", bufs=1) as pool:
            # two pipeline slots (A/B); chunk 2 reuses slot 0
            u_ts = [pool.tile([128, FRAME_A], f16, tag=f"u{i}", name=f"u{i}")
                    for i in (0, 1)]
            r_ts = [pool.tile([128, HP * W + 4], f16, tag=f"r{i}", name=f"r{i}")
                    for i in (0, 1)]
            w_ts = [pool.tile([128, H * W], f16, tag=f"w{i}", name=f"w{i}")
                    for i in (0, 1)]
            cs_ts = [pool.tile([128, 2], f32, tag=f"cs{i}", name=f"cs{i}")
                     for i in (0, 1)]

            # r pad rows (row 0 and row 129) are static -60000; interior
            # rows 1..128 are fully rewritten each step.
            for r_t in r_ts:
                nc.vector.memset(r_t[:, 0:W], NEG)
                nc.vector.memset(r_t[:, (HP - 1) * W:HP * W], NEG)

            def u3(i):
                return u_ts[i][:, 0:FRAME].rearrange(
                    "p (h w) -> p h w", h=HP, w=WP)

            def r3(i):
                return r_ts[i][:, 0:HP * W].rearrange(
                    "p (h w) -> p h w", h=HP, w=W)

            def w3(i):
                return w_ts[i][:, :].rearrange("p (h w) -> p h w", h=H, w=W)

            def load(k, slot):
                if timing_small:
                    k = 0
                rows = slice(k * CHUNK, (k + 1) * CHUNK)
                x_v = x_d[rows, :].rearrange("p (h w) -> p h w", h=HP, w=WP)
                uv = u3(slot)
                nc.sync.dma_start(uv[:, 0:65, :], x_v[:, 0:65, :])
                nc.sync.dma_start(uv[:, 65:130, :], x_v[:, 65:130, :])
                nc.sync.dma_start(cs_ts[slot][:, 0:1], c_d[rows, :])
                nc.sync.dma_start(cs_ts[slot][:, 1:2], f_d[rows, :])

            def step(slot, scale_on_scalar):
                uv, rv, wv = u3(slot), r3(slot), w3(slot)
                u_int = uv[:, 1:129, 1:129]
                r_int = rv[:, 1:129, :]
                # row max3 (reads u col pads; writes r interior rows)
                nc.vector.tensor_max(r_int, uv[:, 1:129, 0:128],
                                     uv[:, 1:129, 2:130])
                nc.vector.tensor_max(r_int, r_int, u_int)
                # col max3 (reads r pad rows; writes w)
                nc.vector.tensor_max(wv, rv[:, 0:128, :], rv[:, 2:130, :])
                nc.vector.tensor_max(wv, wv, r_int)
                # blend: u += c * w  (scale off the DVE critical path when
                # a partner chunk's maxes can run meanwhile)
                if scale_on_scalar and SCALE_MODE == "scalar":
                    nc.scalar.activation(w_ts[slot][:, :], w_ts[slot][:, :],
                                         act.Copy, scale=cs_ts[slot][:, 0:1])
                else:
                    nc.vector.tensor_scalar_mul(w_ts[slot][:, :],
                                                w_ts[slot][:, :],
                                                cs_ts[slot][:, 0:1])
                if USE_GP_ADD and scale_on_scalar:
                    nc.gpsimd.tensor_add(u_int, u_int, wv)
                else:
                    nc.vector.tensor_add(u_int, u_int, wv)

            def step_solo(slot):
                # Solo chunk: no partner work to hide ScalarE/gpsimd latency
                # behind, so split rows 3-ways and pipeline the thirds:
                # DVE runs max ops back-to-back while each third's scale
                # (ScalarE) and blend add (gpsimd) trail behind.
                uv, rv, wv = u3(slot), r3(slot), w3(slot)
                bounds = [1 + (128 * i) // C_SPLIT for i in range(C_SPLIT + 1)]
                parts = list(zip(bounds[:-1], bounds[1:]))

                def rowmax(lo, hi):
                    r_p = rv[:, lo:hi, :]
                    nc.vector.tensor_max(r_p, uv[:, lo:hi, 0:128],
                                         uv[:, lo:hi, 2:130])
                    nc.vector.tensor_max(r_p, r_p, uv[:, lo:hi, 1:129])

                def colmax_blend(lo, hi):
                    w_p = wv[:, lo - 1:hi - 1, :]
                    nc.vector.tensor_max(w_p, rv[:, lo - 1:hi - 1, :],
                                         rv[:, lo + 1:hi + 1, :])
                    nc.vector.tensor_max(w_p, w_p, rv[:, lo:hi, :])
                    if USE_GP_ADD and SCALE_MODE == "scalar":
                        nc.scalar.activation(w_p, w_p, act.Copy,
                                             scale=cs_ts[slot][:, 0:1])
                        nc.gpsimd.tensor_add(uv[:, lo:hi, 1:129],
                                             uv[:, lo:hi, 1:129], w_p)
                    else:
                        nc.vector.tensor_scalar_mul(w_p, w_p,
                                                    cs_ts[slot][:, 0:1])
                        nc.vector.tensor_add(uv[:, lo:hi, 1:129],
                                             uv[:, lo:hi, 1:129], w_p)

                # interleave: col max of part i needs row max of part i+1
                rowmax(*parts[0])
                for i in range(1, C_SPLIT):
                    rowmax(*parts[i])
                    colmax_blend(*parts[i - 1])
                colmax_blend(*parts[-1])

            def finish(k, slot, on_gp=False):
                # avgpool 2x2 + per-channel scale, result into w slot
                eng = nc.gpsimd if on_gp else nc.vector
                if timing_small:
                    k = 0
                rows = slice(k * CHUNK, (k + 1) * CHUNK)
                uv, rv, wv = u3(slot), r3(slot), w3(slot)
                # horizontal pairs: q[h, j] = u[h, 2j+1] + u[h, 2j+2]
                u4 = u_ts[slot][:, WP:WP + H * WP].rearrange(
                    "p (h w2 two) -> p h w2 two", h=H, w2=WP // 2, two=2)
                q = rv[:, 1:129, 0:64]
                eng.tensor_add(q, u4[:, :, 0:64, 1:2], u4[:, :, 1:65, 0:1])
                # vertical pairs
                q5 = r_ts[slot][:, W:W + H * W].rearrange(
                    "p (h2 two w) -> p h2 two w", h2=H // 2, two=2, w=W)
                z = wv[:, 0:64, 0:64]
                eng.tensor_add(z, q5[:, :, 0:1, 0:64], q5[:, :, 1:2, 0:64])
                nc.vector.tensor_scalar_mul(z, z, cs_ts[slot][:, 1:2])
                nc.sync.dma_start(
                    y_d[rows, :].rearrange("p (h w) -> p h w", h=64, w=64), z)

            loop_cm = (tc.For_i(0, loop_reps) if loop_reps > 1
                       else contextlib.nullcontext())
            with loop_cm:
                load(0, 0)
                load(1, 1)
                for _ in range(T):
                    step(0, True)
                    step(1, True)
                finish(0, 0, on_gp=FINISH_GP)
                load(2, 0)
                if C_SPLIT > 1:
                    for _ in range(T):
                        step_solo(0)
                else:
                    for _ in range(T):
                        step(0, False)
                finish(1, 1, on_gp=FINISH_GP)
                finish(2, 0)

    nc.compile()
    return nc


def _get_program():
    if "nc" not in _CACHE:
        _CACHE["nc"] = _build_program()
    return _CACHE["nc"]


def pack_inputs(x, pool_strength):
    """Host-side prep: per-core fp16 padded frames + per-channel scales."""
    x = np.asarray(x, dtype=np.float32)
    s = np.asarray(pool_strength, dtype=np.float64).reshape(C)
    c_ch = (s / (1.0 - s)).astype(np.float32)                  # [C]
    f_ch = (((1.0 - s) ** T) * 0.25).astype(np.float32)        # [C]
    cvec = np.ascontiguousarray(np.tile(c_ch, B_PER_CORE)[:, None])  # [384,1]
    fvec = np.ascontiguousarray(np.tile(f_ch, B_PER_CORE)[:, None])

    in_maps = []
    for j in range(N_CORES):
        xj = x[j * B_PER_CORE:(j + 1) * B_PER_CORE].reshape(IMGS, H, W)
        pad = np.full((IMGS, HP, WP), NEG, dtype=np.float16)
        pad[:, 1:129, 1:129] = xj.astype(np.float16)
        in_maps.append({"x": np.ascontiguousarray(pad.reshape(IMGS, FRAME)),
                        "cvec": cvec, "fvec": fvec})
    return in_maps


def kernel(x: np.ndarray, pool_strength: np.ndarray) -> np.ndarray:
    from concourse.bass_utils import run_bass_kernel_spmd

    nc = _get_program()
    in_maps = pack_inputs(x, pool_strength)
    res = run_bass_kernel_spmd(nc, in_maps, list(range(N_CORES)))

    out = np.empty((B, C, H // 2, W // 2), dtype=np.float32)
    for j in range(N_CORES):
        yj = res.results[j]["y"].astype(np.float32).reshape(
            B_PER_CORE, C, H // 2, W // 2)
        out[j * B_PER_CORE:(j + 1) * B_PER_CORE] = yj
    return out
